# revision 5
# baseline (speedup 1.0000x reference)
"""Trainium2 Bass kernel: dense transformer attention layer, TP over heads on 8 cores.

v2 strategy (wire-byte minimized for the axon per-call transfer cost):
  - Weights, rope tables, mask patterns and the ones-vector are baked into the
    NEFF as Const tensors (inline_tensor) — they ship once at model load, not
    per execution. Each core selects its weight shard from the [8, ...] const
    via partition-conditional DMAs (cond=, skipped-DMA semantics).
  - x ships feature-sharded ([512, T] bf16 per core) and is AllGather'd on
    device into a Shared DRAM tile; per-call wire traffic is ~4MB in + 4MB out.
  - Output is written bf16 (host converts to f32).
  - Compute phases unchanged from v1: QKV+RoPE spill, per-(batch,head)
    attention in transposed layout with bounded-score exp (no max pass),
    per-batch y AllGather overlapped with the next batch, then the
    [DPC, T] slice of the output projection.
"""

import sys
import math
import hashlib
import numpy as np

for _p in ("/opt/trn_rl_repo",):
    if _p not in sys.path:
        sys.path.insert(0, _p)

import ml_dtypes  # noqa: E402

import concourse.bass as bass  # noqa: E402
import concourse.mybir as mybir  # noqa: E402
import concourse.tile as tile  # noqa: E402
from concourse import bacc  # noqa: E402
from concourse.bass_utils import run_bass_kernel_spmd  # noqa: E402

BF16 = mybir.dt.bfloat16
F32 = mybir.dt.float32
BF16NP = ml_dtypes.bfloat16

B, L, NH, HD = 2, 2048, 32, 128
C = NH * HD              # 4096
T = B * L                # 4096 tokens total
NCORES = 8
DPC = C // NCORES        # 512 dims per core
HPC = DPC // HD          # 4 heads per core
FO = C // 128            # 32 feature blocks (contraction)
TN1 = 512                # token block for projection phases
NB1 = T // TN1           # 8
QBS = 512                # q block for attention
QB = L // QBS            # 4 per batch
KTILES = L // 128        # 16 k tiles per batch
SCALE = 1.0 / math.sqrt(HD)

_CACHED = {}


def _build(maskT_bool, consts, dist=True):
    """maskT_bool: [L, L] bool, maskT[k, q] = attend(q -> k).
    consts: host-prepped arrays baked into the NEFF."""
    nc = bacc.Bacc("TRN2", target_bir_lowering=False, debug=False,
                   num_devices=NCORES)

    xs_d = nc.dram_tensor("xs", [DPC, T], BF16, kind="ExternalInput")
    out_d = nc.dram_tensor("out", [DPC, T], BF16, kind="ExternalOutput")

    wqk_c = nc.inline_tensor(consts["wqk_all"], name="wqk_all")
    wv_c = nc.inline_tensor(consts["wv_all"], name="wv_all")
    wo4_c = nc.inline_tensor(consts["wo4_all"], name="wo4_all")
    cos_c = nc.inline_tensor(consts["cos2"], name="cos2c")
    sin_c = nc.inline_tensor(consts["sin2"], name="sin2c")
    ones_c = nc.inline_tensor(consts["ones"], name="onesc")
    periodic = consts["mask4"] is not None
    if periodic:
        mask_c = nc.inline_tensor(consts["mask4"], name="mask4c")
    else:
        mask_c = nc.inline_tensor(consts["maskt"], name="masktc")

    # classify attention tiles: 0 skip, 1 mixed (needs mask), 2 full
    cls = np.zeros((KTILES, QB), np.int8)
    for kt in range(KTILES):
        for qb in range(QB):
            m = maskT_bool[kt * 128:(kt + 1) * 128, qb * QBS:(qb + 1) * QBS]
            cls[kt, qb] = 0 if not m.any() else (2 if m.all() else 1)

    Exp = mybir.ActivationFunctionType.Exp

    with tile.TileContext(nc) as tc, nc.allow_low_precision(
            reason="bf16 rope temps / softmax-normalizer broadcast / bf16 "
                   "output; rel-err budget is 2e-2 and matmul accumulation "
                   "stays fp32"):
        nc.cache_partition_id()
        pid = nc.partition_id()

        def dyn_core(ap0, stride):
            """Core-0 AP shifted by partition_id * stride elements — selects
            this core's slice of an [NCORES, ...] const with one DMA."""
            return bass.AP(tensor=ap0.tensor, offset=ap0.offset + pid * stride,
                           ap=ap0.ap, dep_tracking_offset=ap0.offset)
        with (
            tc.tile_pool(name="stage", bufs=2) as stage,
            tc.tile_pool(name="psum", bufs=1, space="PSUM") as psp,
            tc.tile_pool(name="dram", bufs=1, space="DRAM") as dram,
        ):
            xt_chunk = [dram.tile([C, L], BF16, addr_space="Shared",
                                  name=f"xt_chunk{b}") for b in range(B)]
            qk_d = dram.tile([2 * DPC, T], BF16)      # Q rows, then K rows
            v_d = dram.tile([T, DPC], BF16)           # token-major V
            y_loc = [dram.tile([DPC, L], BF16, name=f"y_loc{b}")
                     for b in range(B)]
            y_full = [dram.tile([C, L], BF16, addr_space="Shared",
                                name=f"y_full{b}") for b in range(B)]

            # ---- phase 0: all-gather x feature shards, one chunk per batch
            # so phase-1 compute on batch 0 overlaps batch 1's gather
            # (collectives cannot read IO tensors, so stage the shard into
            # Internal DRAM tiles first)
            xs_stage = [dram.tile([DPC, L], BF16, name=f"xs_stage{b}")
                        for b in range(B)]
            for b in range(B):
                nc.sync.dma_start(xs_stage[b][:], xs_d[:, b * L:(b + 1) * L])
                if dist:
                    nc.gpsimd.collective_compute(
                        "AllGather", mybir.AluOpType.bypass,
                        ins=[xs_stage[b][:].opt()],
                        outs=[xt_chunk[b][:].opt()],
                        replica_groups=[list(range(NCORES))],
                    )
                else:
                    for c in range(NCORES):
                        nc.scalar.dma_start(
                            xt_chunk[b][c * DPC:(c + 1) * DPC, :],
                            xs_stage[b][:])

            xt_rc = [xt_chunk[b][:].rearrange("(fo p) t -> p fo t", p=128)
                     for b in range(B)]
            v_r = v_d[:].rearrange("(kt p) d -> p kt d", p=128)

            with (
                tc.tile_pool(name="wres", bufs=1) as wres,
                tc.tile_pool(name="xs", bufs=5) as xsp,
                tc.tile_pool(name="kvp", bufs=3) as kvp,
                tc.tile_pool(name="ptp", bufs=4) as ptp,
            ):
                # ---- phase 1 setup: weight/const loads first (independent
                # of the x gather, so they stream while it runs)
                GF = 8            # fo per x chunk
                NG = FO // GF     # 4 chunks per token block

                w_mb = []
                wqk_stride = 2 * HPC * 128 * FO * 128
                for mb in range(2 * HPC):
                    t = wres.tile([128, FO, 128], BF16, name=f"wmb{mb}")
                    nc.sync.dma_start(
                        t[:], dyn_core(wqk_c[0, mb].rearrange(
                            "p (fo j) -> p fo j", j=128), wqk_stride))
                    w_mb.append(t)
                w_v = wres.tile([128, FO, DPC], BF16)
                nc.sync.dma_start(
                    w_v[:], dyn_core(wv_c[0].rearrange(
                        "p (fo j) -> p fo j", j=DPC), 128 * FO * DPC))
                ones_sb = wres.tile([128, 1], BF16)
                nc.sync.dma_start(ones_sb[:], ones_c[:, :])
                if periodic:
                    mask_sb = wres.tile([128, 4, QBS], BF16)
                    nc.sync.dma_start(mask_sb[:], mask_c[:, :, :])

                NBB = NB1 // B    # token blocks per batch chunk

                def load_x(n):
                    tsl = slice(n * TN1, (n + 1) * TN1)
                    csl = slice((n % NBB) * TN1, (n % NBB + 1) * TN1)
                    xc = []
                    for g in range(NG):
                        xg = xsp.tile([128, GF, TN1], BF16, tag="xchunk",
                                      name=f"xg{n}_{g}")
                        nc.sync.dma_start(
                            xg[:], xt_rc[n // NBB][:, g * GF:(g + 1) * GF, csl])
                        xc.append(xg)
                    cos_sb = stage.tile([128, TN1], BF16, tag="cosl", bufs=2,
                                        name=f"cos{n}")
                    nc.sync.dma_start(cos_sb[:], cos_c[:, tsl])
                    sin_sb = stage.tile([128, TN1], BF16, tag="sinl", bufs=2,
                                        name=f"sin{n}")
                    nc.sync.dma_start(sin_sb[:], sin_c[:, tsl])
                    return xc, cos_sb, sin_sb

                def p1_block(n):
                    tsl = slice(n * TN1, (n + 1) * TN1)
                    xc, cos_sb, sin_sb = load_x(n)
                    for mb in range(2 * HPC):
                        ps = psp.tile([128, TN1], F32, tag="mm", bufs=4)
                        for fo in range(FO):
                            nc.tensor.matmul(ps[:], w_mb[mb][:, fo],
                                             xc[fo // GF][:, fo % GF],
                                             start=(fo == 0), stop=(fo == FO - 1))
                        # rope: out = p*cos2 + rot(p)*sin2 (sin2 top half negated)
                        tmp = stage.tile([128, TN1], BF16, tag="ropetmp")
                        rot = stage.tile([128, TN1], BF16, tag="roperot")
                        nc.vector.tensor_mul(tmp[:], ps[:], cos_sb[:])
                        nc.vector.tensor_mul(rot[0:64], ps[64:128], sin_sb[0:64])
                        nc.vector.tensor_mul(rot[64:128], ps[0:64], sin_sb[64:128])
                        qh = stage.tile([128, TN1], BF16, tag="qkout")
                        nc.vector.tensor_add(qh[:], tmp[:], rot[:])
                        nc.sync.dma_start(qk_d[mb * 128:(mb + 1) * 128, tsl], qh[:])
                    for tb in range(TN1 // 128):
                        psv = psp.tile([128, DPC], F32, tag="acc", bufs=2)
                        for fo in range(FO):
                            nc.tensor.matmul(
                                psv[:], xc[fo // GF][:, fo % GF, tb * 128:(tb + 1) * 128],
                                w_v[:, fo], start=(fo == 0), stop=(fo == FO - 1))
                        vh = stage.tile([128, DPC], BF16, tag="vout")
                        nc.any.tensor_copy(vh[:], psv[:])
                        nc.sync.dma_start(
                            v_d[n * TN1 + tb * 128:n * TN1 + (tb + 1) * 128, :], vh[:])

                # ---- phase 2: attention per (batch, head).
                # V is loaded once per batch ([128, 16, 512], 1KB descriptors)
                # and sliced per head in SBUF instead of 256B-descriptor
                # per-head gathers.
                def load_k(b, hb):
                    bsl = slice(b * L, (b + 1) * L)
                    k_sb = kvp.tile([128, L], BF16, tag="katt", bufs=2,
                                    name=f"k{b}_{hb}")
                    nc.sync.dma_start(
                        k_sb[:], qk_d[DPC + hb * 128:DPC + (hb + 1) * 128, bsl])
                    return k_sb

                def load_vb(b):
                    vb = kvp.tile([128, KTILES, DPC], BF16, tag="vbatch",
                                  bufs=1, name=f"vb{b}")
                    nc.sync.dma_start(
                        vb[:], v_r[:, b * KTILES:(b + 1) * KTILES, :])
                    return vb

                def p2_head(b, hb, k_sb, vb):
                    for qb in range(QB):
                        acts = [kt for kt in range(KTILES) if cls[kt, qb] > 0]
                        q_sb = kvp.tile([128, QBS], BF16, tag="qatt")
                        nc.sync.dma_start(
                            q_sb[:], qk_d[hb * 128:(hb + 1) * 128,
                                          b * L + qb * QBS:b * L + (qb + 1) * QBS])
                        y_ps = psp.tile([128, QBS], F32, tag="acc", bufs=2)
                        rs_ps = psp.tile([1, QBS], F32, tag="rs", bufs=2)
                        for i, kt in enumerate(acts):
                            st = psp.tile([128, QBS], F32, tag="mm", bufs=4)
                            nc.tensor.matmul(st[:], k_sb[:, kt * 128:(kt + 1) * 128],
                                             q_sb[:], start=True, stop=True)
                            pt = ptp.tile([128, QBS], BF16, tag="pt")
                            nc.scalar.activation(pt[:], st[:], Exp, scale=SCALE)
                            if cls[kt, qb] == 1:
                                if periodic:
                                    mt = mask_sb[:, kt - 4 * qb]
                                else:
                                    mtt = ptp.tile([128, QBS], BF16,
                                                   tag="mtile", bufs=2)
                                    nc.sync.dma_start(
                                        mtt[:], mask_c[kt * 128:(kt + 1) * 128,
                                                       qb * QBS:(qb + 1) * QBS])
                                    mt = mtt[:]
                                nc.vector.tensor_mul(pt[:], pt[:], mt)
                            first, last = (i == 0), (i == len(acts) - 1)
                            nc.tensor.matmul(rs_ps[:], ones_sb[:], pt[:],
                                             start=first, stop=last)
                            nc.tensor.matmul(
                                y_ps[:], vb[:, kt, hb * 128:(hb + 1) * 128],
                                pt[:], start=first, stop=last)
                        rinv = stage.tile([1, QBS], BF16, tag="rinv", bufs=2)
                        nc.vector.reciprocal(rinv[:], rs_ps[:])
                        rb = stage.tile([128, QBS], BF16, tag="rbc")
                        nc.gpsimd.partition_broadcast(rb[:], rinv[:])
                        y_sb = stage.tile([128, QBS], BF16, tag="yout")
                        nc.vector.tensor_mul(y_sb[:], y_ps[:], rb[:])
                        nc.sync.dma_start(
                            y_loc[b][hb * 128:(hb + 1) * 128,
                                     qb * QBS:(qb + 1) * QBS], y_sb[:])

                # ---- interleaved emission: P1(b) -> P2(b) -> y-gather(b),
                # so each batch's y all-gather (and the second x-gather
                # chunk) overlaps the next batch's projection/attention
                # compute on the in-order engine queues.
                for b in range(B):
                    for n in range(b * NBB, (b + 1) * NBB):
                        p1_block(n)
                    vb = load_vb(b)
                    k_cur = load_k(b, 0)
                    for hb in range(HPC):
                        k_sb = k_cur
                        if hb + 1 < HPC:
                            k_cur = load_k(b, hb + 1)
                        p2_head(b, hb, k_sb, vb)
                    if dist:
                        nc.gpsimd.collective_compute(
                            "AllGather", mybir.AluOpType.bypass,
                            ins=[y_loc[b].opt()], outs=[y_full[b].opt()],
                            replica_groups=[list(range(NCORES))],
                        )
                    else:
                        # stand-in for the collective: keep it
                        # off the sync/gpsimd queues like TOPSP
                        nc.scalar.dma_start(y_full[b][0:DPC, :],
                                            y_loc[b][:])

                # ---- phase 3: output projection slice [DPC, T]
                wo_t = []
                wo_stride = HPC * 128 * FO * 128
                for mb in range(HPC):
                    t3 = wres.tile([128, FO, 128], BF16, name=f"wmb{mb}")
                    # scalar queue: off the Pool queue so the load isn't
                    # serialized behind the y all-gathers; the name-alias WAR
                    # on wmb releases once phase 1's last matmul retires
                    nc.scalar.dma_start(
                        t3[:], dyn_core(wo4_c[0, mb].rearrange(
                            "p (fo j) -> p fo j", j=128), wo_stride))
                    wo_t.append(t3)
                yf_rs = [yf[:].rearrange("(fo p) t -> p fo t", p=128)
                         for yf in y_full]
                NB3 = L // TN1
                for bb in range(B):
                    for n in range(NB3):
                        tsl = slice(n * TN1, (n + 1) * TN1)
                        yc = []
                        for g in range(NG):
                            yg = xsp.tile([128, GF, TN1], BF16, tag="xchunk",
                                          name=f"yg{bb}_{n}_{g}")
                            nc.sync.dma_start(
                                yg[:], yf_rs[bb][:, g * GF:(g + 1) * GF, tsl])
                            yc.append(yg)
                        for mb in range(DPC // 128):
                            po = psp.tile([128, TN1], F32, tag="mm", bufs=4)
                            for fo in range(FO):
                                nc.tensor.matmul(po[:], wo_t[mb][:, fo],
                                                 yc[fo // GF][:, fo % GF],
                                                 start=(fo == 0),
                                                 stop=(fo == FO - 1))
                            ot = stage.tile([128, TN1], BF16, tag="oout")
                            nc.any.tensor_copy(ot[:], po[:])
                            nc.sync.dma_start(
                                out_d[mb * 128:(mb + 1) * 128,
                                      bb * L + n * TN1:bb * L + (n + 1) * TN1],
                                ot[:])

    nc.compile()
    return nc


def _prep_consts(rope, mask_b, wq, wk, wv, wo):
    """Host-side packing of all NEFF-constant data."""
    rope = np.asarray(rope, np.float32)
    wq = np.asarray(wq, np.float32)
    wk = np.asarray(wk, np.float32)
    wv = np.asarray(wv, np.float32)
    wo = np.asarray(wo, np.float32)

    # rope half-split permutation of q/k output dims
    i = np.arange(HD // 2)
    perm = np.zeros(C, np.int64)
    for h in range(NH):
        perm[h * HD + i] = h * HD + 2 * i
        perm[h * HD + HD // 2 + i] = h * HD + 2 * i + 1
    wq_p, wk_p = wq[perm], wk[perm]

    cos = rope[:, :, 0].T                      # [64, L]
    sin = rope[:, :, 1].T
    cos1 = np.concatenate([cos, cos], 1)       # [64, T]
    sin1 = np.concatenate([sin, sin], 1)
    cos2 = np.ascontiguousarray(np.vstack([cos1, cos1])).astype(BF16NP)
    sin2 = np.ascontiguousarray(np.vstack([-sin1, sin1])).astype(BF16NP)
    maskT = np.ascontiguousarray(mask_b.T)
    ones = np.ones((128, 1), BF16NP)

    wqk_all = np.empty((NCORES, 2 * HPC, 128, FO * 128), BF16NP)
    wv_all = np.empty((NCORES, 128, FO * DPC), BF16NP)
    wo4_all = np.empty((NCORES, HPC, 128, FO * 128), BF16NP)
    for c in range(NCORES):
        sl = slice(c * DPC, (c + 1) * DPC)
        A = np.concatenate([wq_p[sl], wk_p[sl]], 0).T          # [C, 1024]
        wqk_all[c] = (A.reshape(FO, 128, 2 * HPC, 128)
                      .transpose(2, 1, 0, 3)
                      .reshape(2 * HPC, 128, FO * 128)).astype(BF16NP)
        Bv = wv[sl].T                                           # [C, 512]
        wv_all[c] = (Bv.reshape(FO, 128, DPC).transpose(1, 0, 2)
                     .reshape(128, FO * DPC)).astype(BF16NP)
        Aw = wo[sl].T                                           # [C, 512]
        wo4_all[c] = (Aw.reshape(FO, 128, HPC, 128).transpose(2, 1, 0, 3)
                      .reshape(HPC, 128, FO * 128)).astype(BF16NP)

    # periodic causal fast path: mask tiles along the block diagonal repeat
    causal = np.tril(np.ones((L, L), bool))
    if mask_b.shape == (L, L) and bool((mask_b == causal).all()):
        mask4 = np.ascontiguousarray(
            maskT[0:4 * 128, 0:QBS].reshape(4, 128, QBS)
            .transpose(1, 0, 2)).astype(BF16NP)
        maskt = None
    else:
        mask4 = None
        maskt = np.ascontiguousarray(maskT).astype(BF16NP)

    return {
        "wqk_all": wqk_all, "wv_all": wv_all, "wo4_all": wo4_all,
        "cos2": cos2, "sin2": sin2, "ones": ones,
        "mask4": mask4, "maskt": maskt,
    }


def _make_runner(nc, donate=True):
    """Build a reusable jitted runner for nc.

    The bass2jax lowering mutates nc (Const allocations become HLO constants),
    so lowering must happen exactly once per build — we cache this runner, not
    nc. The output buffer is donated and ping-ponged back in as the NEFF's
    in-out operand, so steady-state per-call traffic is the xs shard in and
    the out slice back.
    """
    import time
    import jax
    from jax.experimental.shard_map import shard_map
    from jax.sharding import Mesh, PartitionSpec, NamedSharding
    from concourse import bass2jax as b2j

    b2j.install_neuronx_cc_hook()
    n_cores = NCORES
    partition_name = (nc.partition_id_tensor.name
                      if nc.partition_id_tensor else None)
    out_aval = jax.core.ShapedArray((DPC, T), BF16NP)
    all_in = ["xs", "out"]
    if partition_name is not None:
        all_in.append(partition_name)

    def _body(*args):
        operands = list(args)
        if partition_name is not None:
            operands.append(b2j.partition_id_tensor())
        outs = b2j._bass_exec_p.bind(
            *operands,
            out_avals=(out_aval,),
            in_names=tuple(all_in),
            out_names=("out",),
            lowering_input_output_aliases=(((0, 1),) if donate else ()),
            sim_require_finite=True,
            sim_require_nnan=True,
            nc=nc,
        )
        return tuple(outs)

    devices = jax.devices()[:n_cores]
    mesh = Mesh(np.asarray(devices), ("core",))
    in_specs = (PartitionSpec("core"),) * 2
    out_specs = (PartitionSpec("core"),)
    sharded = jax.jit(shard_map(_body, mesh=mesh, in_specs=in_specs,
                                out_specs=out_specs, check_rep=False),
                      keep_unused=True,
                      donate_argnums=((1,) if donate else ()))
    sh = NamedSharding(mesh, PartitionSpec("core"))

    def run(xs_all, iters):
        """xs_all: np [n_cores*DPC, T] bf16. Returns (out [C, T] bf16, best_ns)."""
        dev_x = jax.device_put(xs_all, sh)
        out_buf = jax.device_put(
            np.zeros((n_cores * DPC, T), BF16NP), sh)
        (out_buf,) = sharded(dev_x, out_buf)
        jax.block_until_ready(out_buf)
        best = None
        for _ in range(max(0, iters - 1)):
            t0 = time.perf_counter()
            (out_buf,) = sharded(dev_x, out_buf)
            jax.block_until_ready(out_buf)
            dt = time.perf_counter() - t0
            best = dt if best is None else min(best, dt)
        out_np = np.asarray(out_buf)      # [n_cores*DPC, T] == [C, T]
        return out_np, (int(best * 1e9) if best is not None else None)

    return run


def kernel(x, rope, mask, max_seq_length, wq, wk, wv, wo, _trace=False,
           _want_results=False):
    x = np.asarray(x, np.float32)
    mask_b = np.asarray(mask, bool)[0, 0]
    maskT_bool = np.ascontiguousarray(mask_b.T)

    h = hashlib.sha256()
    for a in (rope, mask_b, wq, wk, wv, wo):
        h.update(np.ascontiguousarray(a).tobytes())
    key = h.hexdigest()
    run = _CACHED.get(key)
    if run is None:
        consts = _prep_consts(rope, mask_b, wq, wk, wv, wo)
        nc = _build(maskT_bool, consts)
        run = _make_runner(nc)
        _CACHED[key] = run

    xT = np.ascontiguousarray(x.reshape(T, C).T).astype(BF16NP)  # [C, T]
    outT, best_ns = run(xT, iters=(30 if _trace else 1))
    out = np.ascontiguousarray(outT.T).reshape(B, L, C).astype(np.float32)
    if _want_results:
        return out, best_ns
    return out


if __name__ == "__main__":
    print("smoke test build only")
